# revision 55
# baseline (speedup 1.0000x reference)
"""Multi-head causal attention (B=2, S=2048, H=1024, 16 heads) on 8 TRN2
NeuronCores.

Sharding: core c in 0..7 handles batch b = c // 4 and head group g = c % 4
(heads 4g..4g+3).  Each core computes Q/K/V projections for its 4 heads and
causal attention; normalized attention outputs are exchanged with an 8-wide
fp16 AllToAll so core 4b+r ends up with all 16 heads for seq rows
512s+128r of each strip s, out-projects them locally through the full Wo,
and writes fp32 output rows; the host concatenates the chunks.

Device dataflow (per core, all matmuls in fp16 — full PE rate, prefetchable
LDWEIGHTS, ~5e-4 relative error):
  - activations pre-transposed + fp16-cast on host to [1024, 2048]
    (feature-major) since the PE contracts over the partition dim
  - processing is strip-at-a-time (4 q-strips of 512): projection of strip 0
    up front, then causal attention per strip with the NEXT strip's
    projection emitted as 8 deferrable pieces woven between attention
    j-steps (the engines are in-order, so interleaving must be done at
    emission time) to fill exp-latency PE gaps and keep the PE HAM-warm
  - scores computed transposed (scoresT[k, q]) so the exp'd tiles feed the
    attention-value matmul directly as the moving operand, no transposes;
    the two heads of a pair sit in PE row groups 0-63/64-127 and their
    64-contraction matmuls run concurrently (tile_position row tiling)
  - the two heads' score tiles land in one [128, 2, 512] 2-bank PSUM tile so
    a single 1024-wide ACT exp evacuates both (halves ACT instruction count)
  - causal handled by skipping fully-masked 128x512 blocks, trimming the
    q-window of diagonal blocks to [128*i, 512), and multiplying the
    remaining diagonal pattern with precomputed 0/1 masks (DVE)
  - softmax normalization: rowsum row (partition 64 of the attnV PSUM tile)
    -> DVE evac -> PE K=1 outer-product broadcast -> DVE reciprocal approx
    -> multiply during PSUM evacuation; 1/sqrt(64) folded into the exp scale
  - output path per strip: one AllToAll moves [256 feat, 128 q] OT blocks
    (each core sends its chunk to both batches' rank-r block — the block
    index is the global rank, which is per-core, so SPMD symmetry needs the
    duplicate; receivers fold the two halves with host-provided 0/1 masks),
    then a local out-projection contracts all 1024 features in PSUM.  This
    exchanges 262k elements/strip instead of reducing 524k in the CC ALU
    (the collective engine is element-rate-limited at ~16 G elem/s).
    Exchanges fire as each strip completes; receives are deferred into the
    last attention strip's j-loop so the in-order PE never waits on a
    collective.
"""

import sys

for _p in ("/opt/trn_rl_repo", "/root/.axon_site/_ro/trn_rl_repo"):
    if _p not in sys.path:
        sys.path.insert(0, _p)

import numpy as np

import concourse.bass as bass
import concourse.tile as tile
from concourse import bacc
import concourse.mybir as mybir

B = 2
S = 2048
HID = 1024
HEADS_PER_CORE = 4
DH = 64  # head dim
HG = HEADS_PER_CORE * DH  # 256: hidden slice per core
N_CORES = 8
GROUP = 4  # cores per batch (reduction group)

F32 = mybir.dt.float32
F16 = mybir.dt.float16
AF = mybir.ActivationFunctionType
ALU = mybir.AluOpType

KT = 128  # contraction tile (partitions)
QS = 512  # q strip width
NQS = S // QS  # 4 q strips
NST = S // KT  # 16 s tiles


def build_nc():
    nc = bacc.Bacc(
        "TRN2", target_bir_lowering=False, debug=False, num_devices=N_CORES
    )

    # per-core inputs (already sharded/transposed/f16-cast by the host)
    xq = nc.dram_tensor("xq", [NQS, 128, 8, QS], F16, kind="ExternalInput").ap()
    xk = nc.dram_tensor("xk", [NQS, 128, 8, QS], F16, kind="ExternalInput").ap()
    xv = nc.dram_tensor("xv", [NQS, 128, 8, QS], F16, kind="ExternalInput").ap()
    wq = nc.dram_tensor("wq", [128, 8, HG], F16, kind="ExternalInput").ap()
    wk = nc.dram_tensor("wk", [128, 8, HG], F16, kind="ExternalInput").ap()
    wv = nc.dram_tensor("wv", [128, 8, HG], F16, kind="ExternalInput").ap()
    wo = nc.dram_tensor("wo", [128, 8, HID], F16, kind="ExternalInput").ap()
    bqk = nc.dram_tensor("bqk", [2, 2, 128, 1], F32, kind="ExternalInput").ap()
    bvb = nc.dram_tensor(
        "bvb", [128, HEADS_PER_CORE, DH], F32, kind="ExternalInput"
    ).ap()
    bob = nc.dram_tensor("bob", [128, HID], F16, kind="ExternalInput").ap()
    msk = nc.dram_tensor("msk", [128, 2, 4, QS], F16, kind="ExternalInput").ap()
    # per-core batch-select masks for the post-AllToAll fold (1.0 on the
    # core's own batch half, 0.0 on the other)
    msel = nc.dram_tensor(
        "msel", [128, 2, 8, 128], F16, kind="ExternalInput"
    ).ap()

    out_chunk = nc.dram_tensor(
        "out_chunk", [S // GROUP, HID], F32, kind="ExternalOutput"
    ).ap()

    # AllToAll exchange buffers, one per strip: 8 blocks of [256 feat, 128 q]
    a2a_in = nc.dram_tensor("a2a_in", [NQS, 8, HG, 128], F16)
    a2a_out = nc.dram_tensor("a2a_out", [NQS, 8, HG, 128], F16)
    a2a3_in = nc.dram_tensor("a2a3_in", [2, 8, 128, 128], F16)
    a2a3_out = nc.dram_tensor("a2a3_out", [2, 8, 128, 128], F16)
    cc_warm_in = nc.dram_tensor("cc_warm_in", [8, 128], F32)
    cc_warm_out = nc.dram_tensor("cc_warm_out", [8, 128], F32)

    groups8 = [[0, 1, 2, 3, 4, 5, 6, 7]]

    with tile.TileContext(nc) as tc:
        with (
            tc.tile_pool(name="wpool", bufs=1) as wpool,
            tc.tile_pool(name="qkv", bufs=1) as qkv,
            tc.tile_pool(name="xs", bufs=2) as xs,
            tc.tile_pool(name="xsb", bufs=2) as xsb,
            tc.tile_pool(name="atp", bufs=5) as atp,
            tc.tile_pool(name="npool", bufs=4) as npool,
            tc.tile_pool(name="osbp", bufs=2) as osbp,
            tc.tile_pool(name="pj", bufs=2, space="PSUM") as pjp,
            tc.tile_pool(name="sp", bufs=2, space="PSUM") as spp,
            tc.tile_pool(name="po", bufs=2, space="PSUM") as pop,
        ):
            # ---- constants / weights ----
            bq_sb = []
            bk_sb = []
            for m in range(2):
                t = wpool.tile([128, 1], F32, tag=f"bq{m}")
                nc.sync.dma_start(t[:], bqk[0, m])
                bq_sb.append(t)
                t = wpool.tile([128, 1], F32, tag=f"bk{m}")
                nc.scalar.dma_start(t[:], bqk[1, m])
                bk_sb.append(t)
            wq_all = wpool.tile([128, 8, HG], F16, tag="wq")
            nc.sync.dma_start(wq_all[:], wq[:])
            wq_sb = [wq_all[:, k] for k in range(8)]
            wk_all = wpool.tile([128, 8, HG], F16, tag="wk")
            nc.scalar.dma_start(wk_all[:], wk[:])
            wk_sb = [wk_all[:, k] for k in range(8)]
            wv_all = wpool.tile([128, 8, HG], F16, tag="wv")
            nc.gpsimd.dma_start(wv_all[:], wv[:])
            wv_sb = [wv_all[:, k] for k in range(8)]
            bv_sb = wpool.tile([128, HEADS_PER_CORE, DH], F32, tag="bvb")
            nc.scalar.dma_start(bv_sb[:], bvb[:])
            bo_sb = wpool.tile([128, HID], F16, tag="bob")
            nc.sync.dma_start(bo_sb[:], bob[:])
            mask_sb = wpool.tile([128, 2, 4, QS], F16, tag="msk")
            msel_sb = wpool.tile([128, 2, 8, 128], F16, tag="msel")
            # full WoT [1024 feat, 1024 out] as 8 feature-tiles (the
            # post-AllToAll out-projection contracts all 1024 features);
            # mask/msel/wo DMAs are emitted between projection strips so the
            # big wo transfer doesn't delay the strip-0 activation loads
            wo_sb = wpool.tile([128, 8, HID], F16, tag="wo")
            # [1, 64] of ones: stationary operand of the rowsum-broadcast
            # outer-product matmul
            ones_sb = wpool.tile([1, DH], F16, tag="ones")

            # ---- persistent activations (per-strip tiles so attention on
            # early strips doesn't depend on later strips' projections) ----
            qt_sb = [
                [qkv.tile([128, QS], F16, tag=f"qt{m}_{s}", name=f"qt{m}_{s}")
                 for s in range(NQS)]
                for m in range(2)
            ]
            kt_sb = [
                [qkv.tile([128, QS], F16, tag=f"kt{m}_{s}", name=f"kt{m}_{s}")
                 for s in range(NQS)]
                for m in range(2)
            ]
            v_sb = [
                qkv.tile([128, HEADS_PER_CORE, DH + 1], F16, tag=f"v{st}",
                         name=f"v{st}")
                for st in range(NST)
            ]
            ot_sb = [
                [qkv.tile([2 * DH, QS], F16, tag=f"ot{p}_{s}", name=f"ot{p}_{s}")
                 for s in range(NQS)]
                for p in range(2)
            ]

            # ---- per-strip projection ----
            filler = []

            def emit_filler(n=1):
                for _ in range(n):
                    if filler:
                        filler.pop(0)()

            def flush_filler():
                emit_filler(len(filler))

            def proj_strip(s, defer=False, defer_v=False, hold=False):
                xts = {}
                for name, xdram, eng in (
                    ("xq", xq, nc.sync),
                    ("xk", xk, nc.scalar),
                    ("xv", xv, nc.gpsimd),
                ):
                    if s == 0:
                        # two-piece loads: first matmuls start after 512 KB
                        for h in range(2):
                            t = xs.tile([128, 4, QS], F16, tag=name,
                                        name=f"{name}{s}_{h}")
                            eng.dma_start(t[:], xdram[s, :, 4 * h : 4 * h + 4])
                            for k in range(4):
                                xts[(name, 4 * h + k)] = t[:, k]
                    else:
                        t = xsb.tile([128, 8, QS], F16, tag=name + "b",
                                     name=f"{name}{s}")
                        eng.dma_start(t[:], xdram[s])
                        for k in range(8):
                            xts[(name, k)] = t[:, k]
                # QT/KT [256, 512]: weight-stationary, k-inner, bias fused in
                # the ACT PSUM->SBUF evacuation; emitted as 8 deferrable
                # pieces so they can be woven between attention j-steps
                def qk_piece(name, w_sb, b_sb, dest, m):
                    ps = pjp.tile([128, QS], F32, tag="pj",
                                  name=f"ps_{name}{s}{m}")
                    for k in range(8):
                        nc.tensor.matmul(
                            ps[:],
                            w_sb[k][:, 128 * m : 128 * m + 128],
                            xts[(name, k)][:],
                            start=(k == 0),
                            stop=(k == 7),
                        )
                    nc.scalar.activation(
                        dest[m][s][:], ps[:], AF.Identity, bias=b_sb[m][:]
                    )

                def v_piece(u):
                    ps = pjp.tile([128, HEADS_PER_CORE, DH], F32, tag="pj",
                                  name=f"ps_v{s}{u}")
                    for k in range(8):
                        nc.tensor.matmul(
                            ps[:],
                            xts[("xv", k)][:, 128 * u : 128 * u + 128],
                            wv_sb[k],
                            start=(k == 0),
                            stop=(k == 7),
                        )
                    st = 4 * s + u
                    nc.vector.tensor_tensor(
                        v_sb[st][:, :, 0:DH], ps[:], bv_sb[:], ALU.add
                    )

                pieces = []
                for name, w_sb, b_sb, dest in (
                    ("xq", wq_sb, bq_sb, qt_sb),
                    ("xk", wk_sb, bk_sb, kt_sb),
                ):
                    for m in range(2):
                        pieces.append(
                            lambda name=name, w_sb=w_sb, b_sb=b_sb,
                            dest=dest, m=m: qk_piece(name, w_sb, b_sb, dest, m)
                        )
                v_pieces = [lambda u=u: v_piece(u) for u in range(4)]
                if hold:
                    # x loads fired above; compute pieces handed back to the
                    # caller to emit once earlier strips' attention is out
                    return pieces + v_pieces
                if defer:
                    filler.extend(pieces + v_pieces)
                else:
                    for p_ in pieces:
                        p_()
                    if defer_v:
                        # V pieces ride the attention weave: piece u lands
                        # just before attnV consumes v_sb[4s+u]
                        filler.extend(v_pieces)
                    else:
                        for p_ in v_pieces:
                            p_()

            # ---- AllToAll exchange + local out-projection per strip ----
            # Each core sends its normalized OT feature-slices for q-chunk r
            # to the rank-r cores of BOTH batches (blocks r and 4+r; block
            # index is the global rank, which is per-core, so SPMD symmetry
            # requires the duplicate).  The receiver folds the two batch
            # halves with per-core 0/1 masks, then out-projects its 128 rows
            # over the full Wo (the partial-sum reduction happens in PSUM,
            # not in the CC ALU: 262k exchanged elements/strip vs 524k
            # reduced for a ReduceScatter).
            def emit_exchange(s):
                for p in range(2):
                    for m in range(2):
                        eng = nc.sync if (p + m) % 2 == 0 else nc.scalar
                        eng.dma_start(
                            a2a_in[
                                s, 4 * m : 4 * m + 4, 128 * p : 128 * p + 128, :
                            ].rearrange("r f q -> f r q"),
                            ot_sb[p][s][:],
                        )
                nc.gpsimd.collective_compute(
                    "AllToAll",
                    ALU.bypass,
                    replica_groups=groups8,
                    ins=[a2a_in[s].opt()],
                    outs=[a2a_out[s].opt()],
                )

            def emit_receive(s):
                og = osbp.tile([128, 16, 128], F16, tag="og", name="og")
                nc.gpsimd.dma_start(
                    og[:], a2a_out[s].rearrange("b (g a) c -> a (b g) c", a=128)
                )
                ogf = osbp.tile([128, 8, 128], F16, tag="ogf", name="ogf")
                nc.vector.tensor_tensor(
                    ogf[:], og[:, 0:8], msel_sb[:, 0], ALU.mult
                )
                ogb = osbp.tile([128, 8, 128], F16, tag="ogb", name="ogb")
                nc.vector.tensor_tensor(
                    ogb[:], og[:, 8:16], msel_sb[:, 1], ALU.mult
                )
                nc.vector.tensor_tensor(ogf[:], ogf[:], ogb[:], ALU.add)
                for eh in range(2):
                    esl = slice(QS * eh, QS * eh + QS)
                    ps = pjp.tile([128, QS], F32, tag="pj", name="psu")
                    for f in range(8):
                        nc.tensor.matmul(
                            ps[:],
                            ogf[:, f, :],
                            wo_sb[:, f, esl],
                            start=(f == 0),
                            stop=(f == 7),
                        )
                    t_o = osbp.tile([128, QS], F32, tag="to", name="to")
                    nc.vector.tensor_tensor(t_o[:], ps[:], bo_sb[:, esl], ALU.add)
                    nc.sync.dma_start(
                        out_chunk[128 * s : 128 * s + 128, esl], t_o[:]
                    )

            # ---- per-strip attention ----
            def normalize(pair, s, pso_t, hh):
                rs = npool.tile([1, QS], F16, tag="rs", name="rs")
                nc.vector.tensor_copy(rs[:], pso_t[DH : DH + 1])
                rbc = pjp.tile([64, QS], F32, tag="pj", name="rbc")
                nc.tensor.matmul(rbc[:], ones_sb[:], rs[:], start=True, stop=True)
                rrec = npool.tile([64, QS], F32, tag="rrec", name="rrec")
                nc.vector.reciprocal_approx_fast(rrec[:], rbc[:])
                nc.vector.tensor_tensor(
                    ot_sb[pair][s][64 * hh : 64 * hh + 64],
                    pso_t[0:DH],
                    rrec[:],
                    ALU.mult,
                )

            pending_exchange = []
            ogf3 = [None, None]

            def emit_exchange3(p):
                # strip 3, one head-pair: half-size AllToAll so pair 0's
                # exchange+fold overlap pair 1's attention
                for m in range(2):
                    eng = nc.sync if m == 0 else nc.scalar
                    eng.dma_start(
                        a2a3_in[p, 4 * m : 4 * m + 4].rearrange(
                            "r f q -> f r q"
                        ),
                        ot_sb[p][3][:],
                    )
                nc.gpsimd.collective_compute(
                    "AllToAll",
                    ALU.bypass,
                    replica_groups=groups8,
                    ins=[a2a3_in[p].opt()],
                    outs=[a2a3_out[p].opt()],
                )

            def emit_fold3(p):
                og = osbp.tile([128, 8, 128], F16, tag=f"og3{p}", name=f"og3{p}")
                nc.gpsimd.dma_start(
                    og[:], a2a3_out[p].rearrange("b (g a) c -> a (b g) c", a=128)
                )
                f = osbp.tile([128, 4, 128], F16, tag=f"ogf3{p}", name=f"ogf3{p}")
                nc.vector.tensor_tensor(
                    f[:], og[:, 0:4], msel_sb[:, 0, 0:4], ALU.mult
                )
                b = osbp.tile([128, 4, 128], F16, tag=f"ogb3{p}", name=f"ogb3{p}")
                nc.vector.tensor_tensor(
                    b[:], og[:, 4:8], msel_sb[:, 1, 0:4], ALU.mult
                )
                nc.vector.tensor_tensor(f[:], f[:], b[:], ALU.add)
                ogf3[p] = f

            def emit_receive3_tail():
                emit_fold3(1)
                for eh in range(2):
                    esl = slice(QS * eh, QS * eh + QS)
                    ps = pjp.tile([128, QS], F32, tag="pj", name="psu3")
                    for p in range(2):
                        for g in range(4):
                            nc.tensor.matmul(
                                ps[:],
                                ogf3[p][:, g, :],
                                wo_sb[:, 2 * g + p, esl],
                                start=(p == 0 and g == 0),
                                stop=(p == 1 and g == 3),
                            )
                    t_o = osbp.tile([128, QS], F32, tag="to", name="to")
                    nc.vector.tensor_tensor(
                        t_o[:], ps[:], bo_sb[:, esl], ALU.add
                    )
                    nc.sync.dma_start(
                        out_chunk[128 * 3 : 128 * 3 + 128, esl], t_o[:]
                    )

            def attn_strip(s):
                nkt = 4 * s + 4
                for pair in range(2):
                    pso = {
                        hh: pop.tile([DH + 1, QS], F32, tag="po",
                                     name=f"pso{s}{pair}{hh}")
                        for hh in range(2)
                    }
                    ats = {}

                    def do_scores(j):
                        i = j - 4 * s
                        off = 128 * i if i > 0 else 0
                        sp = spp.tile([128, 2, QS], F32, tag="sp",
                                      name=f"sp{s}{pair}{j}")
                        for hh in range(2):
                            hp = 64 * hh
                            nc.tensor.matmul(
                                sp[:, hh, off:QS],
                                kt_sb[pair][j // 4][
                                    hp : hp + 64,
                                    128 * (j % 4) : 128 * (j % 4) + 128,
                                ],
                                qt_sb[pair][s][hp : hp + 64, off:QS],
                                start=True,
                                stop=True,
                            )
                        at = atp.tile([128, 2, QS], F16, tag="at",
                                      name=f"at{s}{pair}{j}")
                        nc.scalar.activation(
                            at[:, :, off:QS], sp[:, :, off:QS], AF.Exp,
                            scale=1.0 / 8.0,
                        )
                        if i >= 0:
                            nc.vector.tensor_tensor(
                                at[:, :, off:QS],
                                at[:, :, off:QS],
                                mask_sb[:, :, i, off:QS],
                                ALU.mult,
                            )
                        ats[j] = at

                    do_scores(0)
                    if nkt > 1:
                        do_scores(1)
                    for j in range(nkt):
                        if j + 2 < nkt:
                            do_scores(j + 2)
                        if pair == 0:
                            # exchange for the strip whose normalize was
                            # emitted a k-tile ago; receives deferred two
                            # strips so the PE never waits on the AllToAll
                            if j == 1:
                                for s_r in pending_exchange:
                                    emit_exchange(s_r)
                                pending_exchange.clear()
                            if s == 3 and j in (1, 5, 10):
                                emit_receive(0 if j == 1 else (1 if j == 5 else 2))
                        if pair == 1 and s == 3 and j == 11:
                            emit_fold3(0)
                        emit_filler(1)
                        i = j - 4 * s
                        off = 128 * i if i > 0 else 0
                        for hh in range(2):
                            h = 2 * pair + hh
                            nc.tensor.matmul(
                                pso[hh][:, off:QS],
                                v_sb[j][:, h, :],
                                ats[j][:, hh, off:QS],
                                start=(j == 0),
                                stop=(j == nkt - 1),
                            )
                        del ats[j]
                    for hh in range(2):
                        normalize(pair, s, pso[hh], hh)
                    if s == 3:
                        emit_exchange3(pair)
                if s != 3:
                    pending_exchange.append(s)

            # ---- schedule: proj interleaved between attention strips so
            # the scheduler can fill PE exp-latency gaps with projection
            # matmuls ----
            proj_strip(0, defer_v=True)
            nc.sync.dma_start(mask_sb[:], msk[:])
            nc.scalar.activation(
                ones_sb[:], bo_sb[0:1, 0:DH], AF.Identity, scale=0.0, bias=1.0
            )
            # prefill the ones column of each V tile once (rowsums fall out
            # of the attnV matmul); disjoint from the per-head evac ranges
            for st in range(NST):
                nc.scalar.activation(
                    v_sb[st][:, :, DH], bo_sb[:, 0:HEADS_PER_CORE],
                    AF.Identity, scale=0.0, bias=1.0,
                )
            held = proj_strip(1, hold=True)
            nc.gpsimd.dma_start(msel_sb[:], msel[:])
            nc.gpsimd.dma_start(wo_sb[:], wo[:])
            attn_strip(0)
            filler.extend(held)
            flush_filler()
            proj_strip(2, defer=True)
            attn_strip(1)
            flush_filler()
            proj_strip(3, defer=True)
            attn_strip(2)
            flush_filler()
            attn_strip(3)
            flush_filler()
            for s_r in pending_exchange:
                emit_exchange(s_r)
            emit_receive3_tail()

    nc.compile()
    return nc


_NC = None
_RUNNER = None


def _get_runner():
    """Build the compiled 8-core PJRT callable once and cache it."""
    global _NC, _RUNNER
    if _RUNNER is not None:
        return _RUNNER

    import jax
    import numpy as _np
    from jax.sharding import Mesh, PartitionSpec
    from jax.experimental.shard_map import shard_map
    from concourse.bass2jax import (
        _bass_exec_p,
        install_neuronx_cc_hook,
        partition_id_tensor,
    )

    _NC = build_nc()
    nc = _NC
    install_neuronx_cc_hook()

    partition_name = nc.partition_id_tensor.name if nc.partition_id_tensor else None
    in_names = []
    out_names = []
    out_avals = []
    zero_outs = []
    for alloc in nc.m.functions[0].allocations:
        if not isinstance(alloc, mybir.MemoryLocationSet):
            continue
        name = alloc.memorylocations[0].name
        if alloc.kind == "ExternalInput":
            if name != partition_name:
                in_names.append(name)
        elif alloc.kind == "ExternalOutput":
            shape = tuple(alloc.tensor_shape)
            dtype = mybir.dt.np(alloc.dtype)
            out_names.append(name)
            out_avals.append(jax.core.ShapedArray(shape, dtype))
            zero_outs.append(_np.zeros(shape, dtype))
    n_params = len(in_names)
    n_outs = len(out_avals)
    all_in_names = list(in_names) + list(out_names)
    if partition_name is not None:
        all_in_names.append(partition_name)

    def _body(*args):
        operands = list(args)
        if partition_name is not None:
            operands.append(partition_id_tensor())
        outs = _bass_exec_p.bind(
            *operands,
            out_avals=tuple(out_avals),
            in_names=tuple(all_in_names),
            out_names=tuple(out_names),
            lowering_input_output_aliases=(),
            sim_require_finite=True,
            sim_require_nnan=True,
            nc=nc,
        )
        return tuple(outs)

    devices = jax.devices()[:N_CORES]
    mesh = Mesh(np.asarray(devices), ("core",))
    in_specs = (PartitionSpec("core"),) * (n_params + n_outs)
    out_specs = (PartitionSpec("core"),) * n_outs
    sharded = jax.jit(
        shard_map(
            _body, mesh=mesh, in_specs=in_specs, out_specs=out_specs,
            check_rep=False,
        ),
        keep_unused=True,
    )

    def run(in_maps):
        per_core = [[_np.asarray(m[name]) for name in in_names] for m in in_maps]
        concat_in = [
            _np.concatenate([per_core[c][i] for c in range(N_CORES)], axis=0)
            for i in range(n_params)
        ]
        concat_zeros = [
            _np.zeros((N_CORES * z.shape[0], *z.shape[1:]), z.dtype)
            for z in zero_outs
        ]
        out_arrs = sharded(*concat_in, *concat_zeros)
        return [
            {
                name: _np.asarray(out_arrs[i]).reshape(
                    N_CORES, *out_avals[i].shape
                )[c]
                for i, name in enumerate(out_names)
            }
            for c in range(N_CORES)
        ]

    _RUNNER = run
    return run


def make_in_maps(query, key, value, Wq, bq, Wk, bk, Wv, bv, Wo, bo):
    query = np.asarray(query, dtype=np.float32)
    key = np.asarray(key, dtype=np.float32)
    value = np.asarray(value, dtype=np.float32)
    Wq = np.asarray(Wq, dtype=np.float32)
    bq = np.asarray(bq, dtype=np.float32)
    Wk = np.asarray(Wk, dtype=np.float32)
    bk = np.asarray(bk, dtype=np.float32)
    Wv = np.asarray(Wv, dtype=np.float32)
    bv = np.asarray(bv, dtype=np.float32)
    Wo = np.asarray(Wo, dtype=np.float32)
    bo = np.asarray(bo, dtype=np.float32)

    def xprep(x, b):
        # [S, HID] -> [NQS, 128, 8, QS]: [s, a, k, c] = x.T[128k + a, QS*s + c]
        xt = x[b].T.reshape(8, 128, NQS, QS)
        return np.ascontiguousarray(xt.transpose(2, 1, 0, 3)).astype(np.float16)

    xqT = [xprep(query, b) for b in range(B)]
    xkT = [xprep(key, b) for b in range(B)]
    xvT = [xprep(value, b) for b in range(B)]

    # diagonal-block causal masks: mask[k, hh, i, q] = 1 if q >= k + 128*i
    # (replicated along an hh axis so one DVE op covers a head pair)
    k_idx = np.arange(128)[:, None, None, None]
    i_idx = np.arange(4)[None, None, :, None]
    q_idx = np.arange(QS)[None, None, None, :]
    masks = np.broadcast_to(
        (q_idx >= k_idx + 128 * i_idx), (128, 2, 4, QS)
    ).astype(np.float16)

    bo_b = np.ascontiguousarray(np.broadcast_to(bo, (128, HID))).astype(
        np.float16
    )
    def wprep(w):
        # [1024 in, N out] -> [128, 8, N]: [a, k, c] = w[128k + a, c]
        return np.ascontiguousarray(
            w.reshape(8, 128, -1).transpose(1, 0, 2)
        ).astype(np.float16)

    # full WoT: rows = input features in global concat order (same for all
    # cores); the per-core feature slices arrive in this order via AllToAll
    wo_t = wprep(Wo.T)

    in_maps = []
    for c in range(N_CORES):
        b = c // GROUP
        g = c % GROUP
        hsl = slice(HG * g, HG * g + HG)
        wq_g = wprep(Wq[hsl].T)  # [128, 8, 256]
        wk_g = wprep(Wk[hsl].T)
        wv_g = wprep(Wv[hsl].T)
        bqk_g = np.stack(
            [bq[hsl].reshape(2, 128), bk[hsl].reshape(2, 128)]
        )  # [2, 2, 128]
        bv_b = np.ascontiguousarray(
            np.broadcast_to(bv[hsl], (128, HG)).reshape(128, HEADS_PER_CORE, DH)
        )
        msel_c = np.zeros((128, 2, 8, 128), np.float16)
        msel_c[:, b] = 1.0
        in_maps.append(
            {
                "xq": xqT[b],
                "xk": xkT[b],
                "xv": xvT[b],
                "wq": wq_g,
                "wk": wk_g,
                "wv": wv_g,
                "wo": wo_t,
                "bqk": bqk_g,
                "bvb": bv_b,
                "bob": bo_b,
                "msk": masks,
                "msel": msel_c,
            }
        )
    return in_maps


RS_CHUNKS = [(0, 512), (512, 512), (1024, 512), (1536, 512)]


def assemble_output(results):
    # for RS chunk (r0, rn), core with group rank r holds global rows
    # [r0 + (rn//4)*r, +rn//4) at out_chunk rows [r0//4, +rn//4)
    out = np.empty((B, S, HID), dtype=np.float32)
    for b in range(B):
        for r in range(GROUP):
            chunk = results[GROUP * b + r]["out_chunk"]
            for r0, rn in RS_CHUNKS:
                q = rn // 4
                out[b, r0 + q * r : r0 + q * (r + 1)] = chunk[
                    r0 // 4 : r0 // 4 + q
                ]
    return out


def kernel(**inputs) -> np.ndarray:
    in_maps = make_in_maps(**inputs)
    run = _get_runner()
    results = run(in_maps)
    return assemble_output(results)


if __name__ == "__main__":
    import reference

    inputs = {k: np.asarray(v) for k, v in reference.setup_inputs().items()}
    got = kernel(**inputs)
    want = np.asarray(reference.reference(**inputs))
    err = np.linalg.norm(got - want) / np.linalg.norm(want)
    print("Relative error:", err)
